# revision 16
# baseline (speedup 1.0000x reference)
"""DeepAR (2-layer LSTM, H=512) Trainium2 Bass kernel — v9.

Full-input contract: kernel(**inputs) takes the unsharded inputs from
setup_inputs() and returns the full [512, 64, 2] output.  Internally the
batch (512) is sharded 64-per-core across 8 NeuronCores (data parallel);
LSTM weights are replicated.

Design (HW-validated pieces noted):
  - [128, 256] "stacked" layout: partition p<64 holds batch p / hidden
    dims 0:256, p>=64 holds batch p-64 / dims 256:512.  Every gate matmul
    is a pair of col-tiled M=64 matmuls at tile_position (0,0)/(0,64)
    which the PE runs concurrently (~1.7-1.9x streamed-column throughput,
    measured on HW; 4-way M=32 tiling gives no further gain).
  - Weight columns are host-permuted so each matmul streams 512 contiguous
    columns straight into its psum bank slot (4 matmuls per contraction
    chunk instead of 8).
  - Gate psum: one full bank per gate pair: IF = [i | f], OG = [o | g],
    [128, 512] fp32.  start=True clears has_written for the whole bank on
    the written partitions only (measured), so each (bank, col-half)
    accumulation cycle opens with exactly one start=True matmul.
  - ACT/DVE ops use all 128 partitions: one sigmoid over [128,512] (i,f),
    sigmoid/tanh [128,256] (o, g), tanh [128,256] (c).
  - h is produced in bf16; transposes are full-width [128,128] bf16 PE
    transposes (each yields 2 contraction chunks; transposes from SBUF
    base partition 64 crash the device, hence full-width), bf16 psum.
  - Software-pipelined emission: at iteration t, g1(t) is already complete
    (E(t) and A(t) were emitted in iteration t-1), so the L1 post starts
    immediately; E(t+1)/A(t+1)/bias(t+1)/B(t+1) fill PE time under the
    post chains.  Decode A(t+1) is emitted after the head's m-feedback.
  - Ldweights dedup is tracked per PE column-group, halving LDW count
    (the (IF,c0),(IF,c1),(OG,c0),(OG,c1) slot pattern reuses each
    stationary in the same col group two matmuls later).
"""
import sys

sys.path.insert(0, "/opt/trn_rl_repo")

import numpy as np

import concourse.bass as bass
import concourse.mybir as mybir
from concourse import bass_utils, tile

F32 = mybir.dt.float32
BF16 = mybir.dt.bfloat16
Act = mybir.ActivationFunctionType

B_FULL, TP, TO, F, H = 512, 192, 128, 64, 512
NC = 8
B = B_FULL // NC            # 64 per core
G = 4 * H                   # 2048 gate width
NSLOT = TP + 1              # 193 feature slots (slot t feeds step t)
XCOLS = NSLOT * B           # 12352
HH = H // 2                 # 256

# hT chunk k (hidden dims [128k, 128k+128)) lives at this col offset
CHUNK_OFF = {0: 0, 2: 64, 1: 128, 3: 192}
# weight columns are host-permuted into psum-bank order so each matmul
# streams 512 contiguous columns: [IF-c0, IF-c1, OG-c0, OG-c1] where
# IF = [i | f] and OG = [o | g] (reference gate order is i, g, f, o)
SLOTS = [("IF", 0, 0), ("IF", 1, 512), ("OG", 0, 1024), ("OG", 1, 1536)]


def gate_perm():
    return np.concatenate([
        np.r_[0:256, 1024:1280],       # IF c0: i[0:256], f[0:256]
        np.r_[256:512, 1280:1536],     # IF c1
        np.r_[1536:1792, 512:768],     # OG c0: o[0:256], g[0:256]
        np.r_[1792:2048, 768:1024],    # OG c1
    ])


def ts(i, n):
    return slice(i * n, (i + 1) * n)


def split_excess_waits(nc):
    """Walrus accepts only one sync-wait per hardware instruction. Hoist
    excess waits onto NoOps (same engine) inserted right before."""
    n = 0
    for f in nc.m.functions:
        for blk in f.blocks:
            out = []
            for inst in blk.instructions:
                si = inst.sync_info
                if si is not None and si.on_wait and len(si.on_wait) > 1:
                    waits = list(si.on_wait)
                    for j, w in enumerate(waits[:-1]):
                        nop = mybir.InstNoOp(
                            name=f"{inst.name}-wnop{j}", ins=[], outs=[])
                        nop.engine = inst.engine
                        nop.sync_info = mybir.SyncInfo(on_wait=[w], on_update=[])
                        out.append(nop)
                        n += 1
                    inst.sync_info = mybir.SyncInfo(
                        on_wait=[waits[-1]], on_update=list(si.on_update))
                out.append(inst)
            blk.instructions = out
    return n


def drop_redundant_ldweights(nc):
    """Remove InstLdweights that reload a stationary operand already in the
    PE array.  Tracked per column-group (tile_position[1]): loading one col
    group does not disturb the others, so e.g. the (IF,c0),(IF,c1),(OG,c0),
    (OG,c1) slot pattern needs only the first two loads.  Any transpose
    resets all groups.  Waits/updates on dropped loads survive on a NoOp."""
    n = 0
    for f in nc.m.functions:
        for blk in f.blocks:
            out = []
            group_key = {}
            for inst in blk.instructions:
                if isinstance(inst, mybir.InstLdweights):
                    w = inst.ins[0]
                    tp = inst.tile_position
                    col = tp[1] if tp else 0
                    key = (getattr(w, "memref", None), w.offset, str(w.ap),
                           str(w.dtype), str(inst.perf_mode), str(tp))
                    if group_key.get(col) == key:
                        si = inst.sync_info
                        if si is not None and (si.on_wait or si.on_update):
                            nop = mybir.InstNoOp(
                                name=f"{inst.name}-ldwnop", ins=[], outs=[])
                            nop.engine = inst.engine
                            nop.sync_info = si
                            out.append(nop)
                        n += 1
                        continue
                    group_key[col] = key
                elif isinstance(inst, mybir.InstMatmult):
                    if inst.is_transpose:
                        group_key = {}
                out.append(inst)
            blk.instructions = out
    return n


def build_program(tp=TP, to=TO, split_waits=True, noload=True):
    NSLOT_ = tp + 1
    XCOLS_ = NSLOT_ * B
    TD = tp - to                # decode steps
    nc = bass.Bass("TRN2", target_bir_lowering=False, debug=False,
                   num_devices=NC)

    xyf_d = nc.dram_tensor("xyf_d", [66, XCOLS_], BF16, kind="ExternalInput").ap()
    w1c0_d = nc.dram_tensor("w1c0_d", [66, G], BF16, kind="ExternalInput").ap()
    w1h_d = nc.dram_tensor("w1h_d", [128, 4, G], BF16, kind="ExternalInput").ap()
    w2_d = nc.dram_tensor("w2_d", [128, 8, G], BF16, kind="ExternalInput").ap()
    wmd_d = nc.dram_tensor("wmd_d", [128, 4 * 64], BF16, kind="ExternalInput").ap()
    b2r_d = nc.dram_tensor("b2r_d", [1, G], BF16, kind="ExternalInput").ap()
    bmd_d = nc.dram_tensor("bmd_d", [1, 1], F32, kind="ExternalInput").ap()
    id_d = nc.dram_tensor("id_d", [128, 128], BF16, kind="ExternalInput").ap()
    outmd_d = nc.dram_tensor("outmd_d", [1, TD * 128], F32,
                             kind="ExternalOutput").ap()

    with tile.TileContext(nc) as tc:
        with tc.sbuf_pool(name="const", bufs=1) as cp, \
             tc.sbuf_pool(name="work", bufs=1) as wp, \
             tc.psum_pool(name="ps", bufs=1) as pp:
            # ---- persistent tiles + input DMA ----
            xyf = cp.tile([66, XCOLS_], BF16, name="xyf")
            w1c0 = cp.tile([66, G], BF16, name="w1c0")
            w1h = cp.tile([128, 4, G], BF16, name="w1h")
            w2 = cp.tile([128, 8, G], BF16, name="w2")
            wmd = cp.tile([128, 4 * 64], BF16, name="wmd")
            b2r = cp.tile([1, G], BF16, name="b2r")
            bmd = cp.tile([1, 1], F32, name="bmd")
            identb = cp.tile([128, 128], BF16, name="identb")
            outmd = cp.tile([1, TD * 128], F32, name="outmd")
            ones = cp.tile([1, 64], BF16, name="ones")
            nc.vector.memset(ones[:, :], 1.0)

            # small tensors first: identb gates the first transpose and
            # must not queue behind ~6MB of weights; xyf's first slots feed
            # A(0) immediately, the tail can land while the scan runs
            nc.sync.dma_start(identb[:, :], id_d[:, :])
            nc.sync.dma_start(bmd[:, :], bmd_d[:, :])
            nc.sync.dma_start(b2r[:, :], b2r_d[:, :])
            nc.sync.dma_start(wmd[:, :], wmd_d[:, :])
            nc.sync.dma_start(w1c0[:, :], w1c0_d[:, :])
            xcut = min(16 * B, XCOLS_)
            nc.sync.dma_start(xyf[:, 0:xcut], xyf_d[:, 0:xcut])
            for k in range(4):
                nc.sync.dma_start(w1h[:, k:k + 1, :], w1h_d[:, k:k + 1, :])
            for k in range(4):
                nc.sync.dma_start(w2[:, k:k + 1, :], w2_d[:, k:k + 1, :])
            if xcut < XCOLS_:
                nc.sync.dma_start(xyf[:, xcut:XCOLS_],
                                  xyf_d[:, xcut:XCOLS_])
            for k in range(4, 8):
                nc.sync.dma_start(w2[:, k:k + 1, :], w2_d[:, k:k + 1, :])

            # ---- state tiles (stacked [128, 256] layout) ----
            c1 = cp.tile([128, HH], F32, name="c1")
            c2 = cp.tile([128, HH], F32, name="c2")
            nc.vector.memset(c1[:, :], 0.0)
            nc.vector.memset(c2[:, :], 0.0)

            def gbanks(tag, bufs):
                gIF = pp.tile([128, 2 * HH], F32, name=f"{tag}IF", tag=tag,
                              bufs=bufs)
                gOG = pp.tile([128, 2 * HH], F32, name=f"{tag}OG", tag=tag,
                              bufs=bufs)
                return {"IF": gIF, "OG": gOG}

            def gchunk(g, st, wcols, group_start, group_stop):
                """Emit the 4 col-tiled matmuls of one contraction chunk.

                st: stationary AP [K, 64]; wcols(base) -> moving AP
                [K, 512] of permuted gate-weight columns.  Each matmul owns
                one (bank, col-half): start=True clears that whole bank for
                its partitions."""
                for bank_sel, c, wbase in SLOTS:
                    nc.tensor.matmul(
                        g[bank_sel][ts(c, 64), :], st, wcols(wbase),
                        start=group_start, stop=group_stop,
                        skip_group_check=True)

            def post(g, c_state, h, tag):
                """LSTM cell post: gates psum -> c update -> h (bf16)."""
                aIF = wp.tile([128, 2 * HH], F32, name=f"aIF{tag}",
                              tag=f"aIF{tag}", bufs=2)
                aO = wp.tile([128, HH], F32, name=f"aO{tag}",
                             tag=f"aO{tag}", bufs=2)
                aG = wp.tile([128, HH], F32, name=f"aG{tag}",
                             tag=f"aG{tag}", bufs=2)
                nc.scalar.activation(aIF[:, :], g["IF"][:, :], Act.Sigmoid)
                nc.scalar.activation(aG[:, :], g["OG"][:, HH:2 * HH], Act.Tanh)
                nc.scalar.activation(aO[:, :], g["OG"][:, 0:HH], Act.Sigmoid)
                t1 = wp.tile([128, HH], F32, name=f"t1{tag}", tag=f"t1{tag}",
                             bufs=2)
                t2 = wp.tile([128, HH], F32, name=f"t2{tag}", tag=f"t2{tag}",
                             bufs=2)
                nc.vector.tensor_mul(t1[:, :], aIF[:, 0:HH], aG[:, :])
                nc.vector.tensor_mul(t2[:, :], aIF[:, HH:2 * HH], c_state[:, :])
                nc.vector.tensor_add(c_state[:, :], t1[:, :], t2[:, :])
                tcs = wp.tile([128, HH], F32, name=f"tc{tag}", tag=f"tc{tag}",
                              bufs=2)
                nc.scalar.activation(tcs[:, :], c_state[:, :], Act.Tanh)
                nc.vector.tensor_mul(h[:, :], aO[:, :], tcs[:, :])

            def transp(h, hT):
                """h [128,256] bf16 -> hT [128,256] bf16 chunk layout."""
                trp = pp.tile([128, 1024], BF16, name="trp", tag="trp",
                              bufs=1)
                nc.tensor.transpose(trp[:, 0:128], h[:, 0:128], identb[:, :])
                nc.tensor.transpose(trp[:, 128:256], h[:, 128:256],
                                    identb[:, :])
                nc.vector.tensor_copy(hT[:, 0:128], trp[:, 0:128])
                nc.vector.tensor_copy(hT[:, 128:256], trp[:, 128:256])

            def emit_A(g, t, group_start=False):
                gchunk(g, xyf[0:66, ts(t, 64)],
                       lambda base: w1c0[:, base:base + 512],
                       group_start=group_start, group_stop=True)

            # ---- prologue: complete g1(0)/open g2(0) ----
            g1_cur = gbanks("g1", 4)
            emit_A(g1_cur, 0, group_start=True)
            g2_cur = gbanks("g2", 2)
            gchunk(g2_cur, ones[:, :], lambda base: b2r[:, base:base + 512],
                   group_start=True, group_stop=False)

            # Software-pipelined steady state.  At iteration t entry:
            # g1(t) is complete (E+A emitted in earlier iterations), g2(t)
            # holds bias(t)+B(t).  PE filler for the post chains comes from
            # E(t+1)/A(t+1)/bias(t+1)/B(t+1) of the next step.
            for t in range(tp):
                last = t == tp - 1
                dec = t >= to - 1      # head/m-feedback steps
                # --- L1 post + transpose ---
                h1 = wp.tile([128, HH], BF16, name="h1", tag="h1", bufs=2)
                h1T = wp.tile([128, HH], BF16, name="h1T", tag="h1T", bufs=2)
                post(g1_cur, c1, h1, "1")
                transp(h1, h1T)
                # --- D: L2 h1 part (finishes g2(t)) ---
                for ki, k in enumerate((0, 2, 1, 3)):
                    gchunk(g2_cur, h1T[:, CHUNK_OFF[k]:CHUNK_OFF[k] + 64],
                           lambda base, k=k: w2[:, k:k + 1, base:base + 512],
                           group_start=False, group_stop=(ki == 3))
                # --- E: L1 h part for t+1; A(t+1) for conditioning steps ---
                if not last:
                    g1_pend = gbanks("g1", 4)
                    for ki, k in enumerate((0, 2, 1, 3)):
                        gchunk(g1_pend,
                               h1T[:, CHUNK_OFF[k]:CHUNK_OFF[k] + 64],
                               lambda base, k=k: w1h[:, k:k + 1,
                                                     base:base + 512],
                               group_start=(ki == 0), group_stop=False)
                    if t + 1 < to:
                        emit_A(g1_pend, t + 1)   # PE filler under L2 post
                # --- L2 post + transpose ---
                h2 = wp.tile([128, HH], BF16, name="h2", tag="h2", bufs=2)
                h2T = wp.tile([128, HH], BF16, name="h2T", tag="h2T", bufs=2)
                post(g2_cur, c2, h2, "2")
                transp(h2, h2T)
                # --- bias for step t+1: waits only the g2 gate ACT reads,
                # so it fills PE time while the h2T copies land ---
                if not last:
                    g2_next = gbanks("g2", 2)
                    gchunk(g2_next, ones[:, :],
                           lambda base: b2r[:, base:base + 512],
                           group_start=True, group_stop=False)
                # --- head: m/d (AR feedback + staged outputs) ---
                if dec:
                    mdp = pp.tile([128, 512], F32, name="mdp", tag="mdp",
                                  bufs=1)
                    for ki, k in enumerate((0, 2, 1, 3)):
                        nc.tensor.matmul(
                            mdp[0:64, 0:64], wmd[:, ts(k, 64)],
                            h2T[:, CHUNK_OFF[k]:CHUNK_OFF[k] + 64],
                            start=(ki == 0), stop=(ki == 3),
                            skip_group_check=True)
                    if not last:
                        # m feedback -> feature row 0, slot t+1 (bf16)
                        nc.scalar.activation(xyf[0:1, ts(t + 1, 64)],
                                             mdp[0:1, 0:64], Act.Identity,
                                             bias=bmd[0:1, 0:1], scale=1.0)
                    if t >= to:
                        s = t - to
                        # raw m/d rows -> staging tile (host adds bm/bd)
                        nc.scalar.activation(outmd[0:1, s * 128:s * 128 + 64],
                                             mdp[0:1, 0:64], Act.Identity)
                        nc.scalar.activation(
                            outmd[0:1, s * 128 + 64:s * 128 + 128],
                            mdp[32:33, 0:64], Act.Identity)
                if not last:
                    if t + 1 >= to:
                        emit_A(g1_pend, t + 1)   # decode: waits m(t) write
                    # --- B for step t+1 (bias emitted above) ---
                    for k in (0, 2, 1, 3):
                        gchunk(g2_next, h2T[:, CHUNK_OFF[k]:CHUNK_OFF[k] + 64],
                               lambda base, k=k: w2[:, 4 + k:5 + k,
                                                    base:base + 512],
                               group_start=False, group_stop=False)
                    g1_cur, g2_cur = g1_pend, g2_next

            nc.sync.dma_start(outmd_d[:, :], outmd[:, :])

    if noload:
        drop_redundant_ldweights(nc)
    n = split_excess_waits(nc) if split_waits else 0
    return nc, n


_CACHE = {}


def _get_program():
    if "nc" not in _CACHE:
        _CACHE["nc"] = build_program()[0]
    return _CACHE["nc"]


def make_core_inputs(x, y, W1, b1, W2, b2, Wm, bm, Wd, bd, tp=TP, to=TO):
    """Host-side prep: returns (in_maps list of 8 dicts, scale [512])."""
    import ml_dtypes
    bf16 = ml_dtypes.bfloat16
    NSLOT_ = tp + 1
    XCOLS_ = NSLOT_ * B
    x = np.asarray(x, np.float32)
    y = np.asarray(y, np.float32)
    W1 = np.asarray(W1, np.float32)
    b1 = np.asarray(b1, np.float32)
    W2 = np.asarray(W2, np.float32)
    b2 = np.asarray(b2, np.float32)
    Wm = np.asarray(Wm, np.float32)
    bm = np.asarray(bm, np.float32)
    Wd = np.asarray(Wd, np.float32)
    bd = np.asarray(bd, np.float32)

    scale = 1.0 + np.mean(y[:, 0:to, 0], axis=1)       # [512]
    y_sc = y[:, 0:to, 0] / scale[:, None]              # [512, to]

    b1a = b1.copy()
    b1a[2 * H:3 * H] += 1.0                             # forget-gate +1
    b2a = b2.copy()
    b2a[2 * H:3 * H] += 1.0

    # row layout: 0 = y/m, 1:64 = x[0:63], 64 = ones (bias), 65 = x[63]
    w1c0 = np.empty((66, G), np.float32)
    w1c0[0] = W1[F]                                     # y/m weight row
    w1c0[1:64] = W1[0:F - 1]                            # x weight rows 0..62
    w1c0[64] = b1a                                      # bias row (ones input)
    w1c0[65] = W1[F - 1]                                # x weight row 63

    perm = gate_perm()
    w1c0 = np.ascontiguousarray(w1c0[:, perm])
    w1h = np.ascontiguousarray(
        W1[F + 1:].reshape(4, 128, G).transpose(1, 0, 2)[:, :, perm])
    w2 = np.ascontiguousarray(
        W2.reshape(8, 128, G).transpose(1, 0, 2)[:, :, perm])

    wmd = np.zeros((128, 4, 64), np.float32)
    wmd[:, :, 0] = Wm[:, 0].reshape(4, 128).T
    wmd[:, :, 32] = Wd[:, 0].reshape(4, 128).T
    wmd = np.ascontiguousarray(wmd.reshape(128, 4 * 64))

    b2row = np.ascontiguousarray(b2a.reshape(1, G)[:, perm])
    bmd = np.asarray(bm, np.float32).reshape(1, 1)
    identity = np.eye(128, dtype=np.float32)

    in_maps = []
    for c in range(NC):
        bs = slice(c * B, (c + 1) * B)
        xyf = np.zeros((66, NSLOT_, B), np.float32)
        xyf[0, 1:to, :] = y_sc[bs, 0:to - 1].T          # shifted y feed
        xt = x[bs].transpose(2, 1, 0)                   # [f, t, b]
        xyf[1:64, 0:tp, :] = xt[0:F - 1]                # x rows 0..62
        xyf[65, 0:tp, :] = xt[F - 1]                    # x row 63
        xyf[64, :, :] = 1.0                             # ones / bias row
        in_maps.append({
            "xyf_d": np.ascontiguousarray(
                xyf.reshape(66, XCOLS_)).astype(bf16),
            "w1c0_d": w1c0.astype(bf16),
            "w1h_d": w1h.astype(bf16),
            "w2_d": w2.astype(bf16), "wmd_d": wmd.astype(bf16),
            "b2r_d": b2row.astype(bf16), "bmd_d": bmd,
            "id_d": identity.astype(bf16),
        })
    return in_maps, scale


def postprocess(results, scale, bm, bd, tp=TP, to=TO):
    """results: list of 8 dicts with outmd_d [tp-to, 128] -> [512, tp-to, 2]."""
    bm = float(np.asarray(bm).reshape(-1)[0])
    bd = float(np.asarray(bd).reshape(-1)[0])
    out = np.empty((B_FULL, tp - to, 2), np.float32)
    for c in range(NC):
        r = results[c]["outmd_d"].reshape(tp - to, 128)
        mean_tb = r[:, 0:64] + bm                       # [t, b]
        dpre_tb = r[:, 64:128] + bd
        bs = slice(c * B, (c + 1) * B)
        sc = scale[bs]
        out[bs, :, 0] = (mean_tb * sc[None, :]).T
        disp = np.logaddexp(dpre_tb, 0.0)               # softplus
        out[bs, :, 1] = (disp * np.sqrt(sc)[None, :]).T
    return out


def kernel(x, y, W1, b1, W2, b2, Wm, bm, Wd, bd):
    in_maps, scale = make_core_inputs(x, y, W1, b1, W2, b2, Wm, bm, Wd, bd)
    nc = _get_program()
    res = bass_utils.run_bass_kernel_spmd(nc, in_maps, core_ids=list(range(NC)))
    return postprocess(res.results, scale, bm, bd)
